# revision 1
# baseline (speedup 1.0000x reference)
"""MoE routing kernel for Trainium2 (8 NeuronCores, SPMD).

Math being implemented (faithful to the reference, including its quirks):
  logits = x @ gate_w + gate_b                  # [B,S,E]
  weights = softmax(logits, axis=1)             # softmax over the SEQUENCE axis
  top2 values/indices over experts; only experts 0 and 1 are ever evaluated
  (the reference loops `for ind in range(top_k)` and uses expert `ind`).
  out[t] = c0[t]*eo_0[t] + c1[t]*eo_1[t], where
  eo_e = softmax_D(gelu(x@w1[e]+b1[e]) @ w2[e] + b2[e]) and c_e[t] is the
  top-2 gate weight when expert e is in token t's top-2, else 0.

Sharding strategy: routing + dispatch on host (0.4% of FLOPs). Only tokens
whose top-2 contains expert 0/1 are computed (~25% each). Cores 0-3 handle
expert 0's tokens, cores 4-7 expert 1's, so each core streams only one
expert's weights. The FFN+softmax runs on-device in feature-major layout.
"""

import sys

import numpy as np

sys.path.insert(0, "/opt/trn_rl_repo")

import concourse.bacc as bacc  # noqa: E402
import concourse.bass as bass  # noqa: E402
import concourse.tile as tile  # noqa: E402
from concourse import mybir  # noqa: E402
from concourse.bass_utils import run_bass_kernel_spmd  # noqa: E402

P = 128
D = 1024
F = 4096
NCORES = 8
CHUNK = 512  # max matmul moving free dim (fp32/f32r)
AF = mybir.ActivationFunctionType

_CACHE = {}


def _gating_coeffs(x, gate_w, gate_b):
    """Host replica of the reference gating. Returns c[T,2] float32 where
    c[:,e] is the gate weight if expert e is in the token's top-2 else 0."""
    B, S, _ = x.shape
    x = np.asarray(x, dtype=np.float32)
    logits = x.reshape(B * S, -1) @ np.asarray(gate_w, dtype=np.float32)
    logits = logits.reshape(B, S, -1) + np.asarray(gate_b, dtype=np.float32)
    # softmax over the sequence axis (axis=1), as in the reference
    m = logits.max(axis=1, keepdims=True)
    e = np.exp(logits - m)
    w = e / e.sum(axis=1, keepdims=True)
    wf = w.reshape(B * S, -1)
    # stable argsort of -w == jax.lax.top_k tie semantics (lower index wins)
    top2 = np.argsort(-wf, axis=-1, kind="stable")[:, :2]
    c = np.zeros((B * S, 2), dtype=np.float32)
    for ex in (0, 1):
        sel = (top2 == ex).any(axis=1)
        c[sel, ex] = wf[sel, ex]
    return c


def _round_f32r(a):
    """Round fp32 to the FP32R format (e8m11: RNE to 11 mantissa bits,
    low 12 bits zero), matching walrus' fp32_to_fp32r."""
    u = np.ascontiguousarray(a, dtype=np.float32).view(np.uint32)
    lsb = (u >> 12) & 1
    u = (u + 0x7FF + lsb) & np.uint32(0xFFFFF000)
    return u.view(np.float32)


def _build_nc(n, use_bf16):
    """Bass program for one core: n tokens (multiple of 128), one expert.

    Feature-major layout throughout: activations are [feature_tile(128), token].
      h^T = gelu(w1^T x^T + b1);  z^T = w2^T h^T + b2;  p = exp(z^T)
      s = colsum_D(p) via ones-matmul (interleaved with phase B)
      g = c / s (serial DVE reciprocal on the [1, n] row)
      out^T = p * broadcast(g)

    DMA issue is spread across sequencers: x on Vector/Scalar, w1 on Sync,
    w2 on GpSimd, outputs on Scalar — the per-dma_start issue cost (~0.7us)
    would serialize on a single sequencer otherwise.
    """
    dt = mybir.dt
    # use_bf16: False = all f32r, True = all bf16, "hybrid" = bf16 layer-1
    sdt = dt.bfloat16 if use_bf16 is True else dt.float32r
    sdt_x = dt.bfloat16 if use_bf16 else dt.float32r  # x and w1 (layer 1)
    f32 = dt.float32
    nchunks = (n + CHUNK - 1) // CHUNK
    chunks = []
    off = 0
    while off < n:
        sz = min(CHUNK, n - off)
        chunks.append((off, sz))
        off += sz
    KD, KF = D // P, F // P  # 8, 32
    # psum-group width: psa/psb pools hold 4 banks each (sp and gb borrow
    # idle slots of the same tags late in the kernel)
    mga = max(1, 4 // nchunks)

    nc = bacc.Bacc()
    xT = nc.dram_tensor("xT", [D, n], sdt_x, kind="ExternalInput")
    w1d = nc.dram_tensor("w1", [D, F], sdt_x, kind="ExternalInput")
    w2d = nc.dram_tensor("w2", [F, D], sdt, kind="ExternalInput")
    b1d = nc.dram_tensor("b1t", [P, KF], f32, kind="ExternalInput")
    b2d = nc.dram_tensor("b2t", [P, KD], f32, kind="ExternalInput")
    cd = nc.dram_tensor("c_row", [1, n], f32, kind="ExternalInput")
    outT = nc.dram_tensor("outT", [D, n], sdt, kind="ExternalOutput")

    with tile.TileContext(nc) as tc:
        with (
            tc.tile_pool(name="const", bufs=1) as const,
            tc.tile_pool(name="acts", bufs=1) as acts,
            tc.tile_pool(name="wpool", bufs=8) as wpool,
            tc.tile_pool(name="gp", bufs=1) as gp,
        ):
            ones_f32 = const.tile([P, 1], f32)
            nc.vector.memset(ones_f32[:], 1.0)
            ones_col = const.tile([P, 1], sdt)
            nc.vector.tensor_copy(ones_col[:], ones_f32[:])
            ones_rf = const.tile([1, P], f32)
            nc.vector.memset(ones_rf[:], 1.0)
            ones_row = const.tile([1, P], sdt)
            nc.vector.tensor_copy(ones_row[:], ones_rf[:])
            warm_f = const.tile([P, CHUNK], f32)
            nc.vector.memset(warm_f[:], 0.0)
            warm = const.tile([P, CHUNK], sdt)
            nc.vector.tensor_copy(warm[:], warm_f[:])

            # x tiles on the Scalar issuer; first tile 4-way split
            xs = acts.tile([P, KD * n], sdt_x)
            for k in range(KD):
                nq = 4 if k == 0 else 2
                qs = P // nq
                for q in range(nq):
                    nc.scalar.dma_start(
                        xs[q * qs : (q + 1) * qs, k * n : (k + 1) * n],
                        xT[k * P + q * qs : k * P + (q + 1) * qs, :],
                    )
            b1t = const.tile([P, KF], f32)
            nc.scalar.dma_start(b1t[:], b1d[:])
            b2t = const.tile([P, KD], f32)
            nc.scalar.dma_start(b2t[:], b2d[:])
            c_row = const.tile([1, n], f32)
            nc.gpsimd.dma_start(c_row[:], cd[:])
            h = acts.tile([P, KF * n], sdt)
            p = acts.tile([P, KD * n], sdt)

            ab_pools = tc.tile_pool(name="psa", bufs=4, space="PSUM")
            psa_pool = ab_pools.__enter__()
            ab_pools2 = tc.tile_pool(name="psb", bufs=4, space="PSUM")
            psb_pool = ab_pools2.__enter__()

            # HAM warm-up: keep PE busy while the first x/w DMAs land
            warm_ps = psa_pool.tile([P, CHUNK], f32, tag="psa", name="warm_ps")
            for _ in range(24):
                nc.tensor.matmul(
                    warm_ps[:], warm[:, :P], warm[:], start=True, stop=True
                )
            warm_out = gp.tile([1, 1], f32)
            nc.vector.tensor_copy(warm_out[:], warm_ps[0:1, 0:1])

            def wslab_load(eng, wd, k, mg0, msz, tagname, split, wdt):
                """Load w[k-th 128 rows, mg0*P:(mg0+msz)*P] as one slab."""
                wslab = wpool.tile([P, msz * P], wdt, tag="ws", name=f"{tagname}_{mg0}_{k}")
                for q in range(split):
                    qs = P // split
                    eng.dma_start(
                        wslab[q * qs : (q + 1) * qs, :],
                        wd[k * P + q * qs : k * P + (q + 1) * qs, mg0 * P : (mg0 + msz) * P],
                    )
                return wslab

            # ---- Phase A: h = gelu(w1.T @ x.T + b1) ----
            for mg0 in range(0, KF, mga):
                msz = min(mga, KF - mg0)
                psas = {}
                for mi in range(msz):
                    for ci in range(nchunks):
                        psas[(mi, ci)] = psa_pool.tile(
                            [P, chunks[ci][1]], f32, tag="psa", name=f"psa_{mg0}_{mi}_{ci}"
                        )
                for k in range(KD):
                    if mg0 == 0:
                        eng = nc.sync if k < 4 else nc.gpsimd
                        split = 4 if k == 0 else 2
                    elif mg0 == mga:
                        # second group: still racing the pipe fill; split on sync
                        eng, split = nc.sync, 2
                    else:
                        eng, split = nc.sync, 1
                    wslab = wslab_load(eng, w1d, k, mg0, msz, "w1s", split, sdt_x)
                    for mi in range(msz):
                        for ci, (c0, csz) in enumerate(chunks):
                            nc.tensor.matmul(
                                psas[(mi, ci)][:],
                                wslab[:, mi * P : (mi + 1) * P],
                                xs[:, k * n + c0 : k * n + c0 + csz],
                                start=(k == 0),
                                stop=(k == KD - 1),
                            )
                for mi in range(msz):
                    m = mg0 + mi
                    for ci, (c0, csz) in enumerate(chunks):
                        nc.scalar.activation(
                            h[:, m * n + c0 : m * n + c0 + csz],
                            psas[(mi, ci)][:],
                            AF.Gelu,
                            bias=b1t[:, m : m + 1],
                        )

            # ---- Phase B: p = exp(w2.T @ h + b2); colsum s interleaved ----
            sps = {}
            for ci, (c0, csz) in enumerate(chunks):
                sps[ci] = psa_pool.tile([1, csz], f32, tag="psa", name=f"sp_{ci}")
            for mg0 in range(0, KD, mga):
                msz = min(mga, KD - mg0)
                psbs = {}
                for mi in range(msz):
                    for ci in range(nchunks):
                        psbs[(mi, ci)] = psb_pool.tile(
                            [P, chunks[ci][1]], f32, tag="psb", name=f"psb_{mg0}_{mi}_{ci}"
                        )
                for k in range(KF):
                    wslab = wslab_load(nc.gpsimd, w2d, k, mg0, msz, "w2s", 1, sdt)
                    for mi in range(msz):
                        for ci, (c0, csz) in enumerate(chunks):
                            nc.tensor.matmul(
                                psbs[(mi, ci)][:],
                                wslab[:, mi * P : (mi + 1) * P],
                                h[:, k * n + c0 : k * n + c0 + csz],
                                start=(k == 0),
                                stop=(k == KF - 1),
                            )
                for mi in range(msz):
                    m = mg0 + mi
                    for ci, (c0, csz) in enumerate(chunks):
                        nc.scalar.activation(
                            p[:, m * n + c0 : m * n + c0 + csz],
                            psbs[(mi, ci)][:],
                            AF.Exp,
                            bias=b2t[:, m : m + 1],
                        )
                        # colsum contribution of this D-tile (interleaved)
                        nc.tensor.matmul(
                            sps[ci][:],
                            ones_col[:],
                            p[:, m * n + c0 : m * n + c0 + csz],
                            start=(m == 0),
                            stop=(m == KD - 1),
                        )

            # ---- Phase C: g = c / s; out = p * broadcast(g) ----
            r_sb = gp.tile([1, n], f32)
            for ci, (c0, csz) in enumerate(chunks):
                nc.vector.reciprocal_approx_fast(r_sb[0:1, c0 : c0 + csz], sps[ci][:])
            g_sb = gp.tile([1, n], sdt)
            nc.vector.tensor_mul(g_sb[:], r_sb[:], c_row[:])
            for ci, (c0, csz) in enumerate(chunks):
                gb_ps = psb_pool.tile([P, csz], f32, tag="psb", name=f"gb_{ci}")
                nc.tensor.matmul(
                    gb_ps[:],
                    ones_row[:],
                    g_sb[0:1, c0 : c0 + csz],
                    start=True,
                    stop=True,
                )
                for k in range(KD):
                    nc.vector.tensor_mul(
                        p[:, k * n + c0 : k * n + c0 + csz],
                        p[:, k * n + c0 : k * n + c0 + csz],
                        gb_ps[:],
                    )
                    eng = nc.scalar if k % 2 == 0 else nc.sync
                    for q in range(2):
                        eng.dma_start(
                            outT[k * P + q * 64 : k * P + (q + 1) * 64, c0 : c0 + csz],
                            p[q * 64 : (q + 1) * 64, k * n + c0 : k * n + c0 + csz],
                        )
            ab_pools2.__exit__(None, None, None)
            ab_pools.__exit__(None, None, None)

    nc.finalize()
    return nc


def _get_nc(n, use_bf16):
    key = (n, use_bf16)
    if key not in _CACHE:
        _CACHE[key] = _build_nc(n, use_bf16)
    return _CACHE[key]


def kernel(x, gate_w, gate_b, w1, b1, w2, b2, top_k, use_bf16="hybrid",
           _trace=False, _tmpdir=None):
    x = np.asarray(x)
    B, S, _ = x.shape
    T = B * S
    assert int(top_k) == 2
    c = _gating_coeffs(x, gate_w, gate_b)

    x_f = np.ascontiguousarray(x.reshape(T, D).astype(np.float32))
    idx = [np.nonzero(c[:, ex])[0] for ex in (0, 1)]  # tokens per expert
    per_core = max(
        (len(idx[0]) + 3) // 4, (len(idx[1]) + 3) // 4, 1
    )
    n = ((per_core + P - 1) // P) * P  # padded tokens per core

    import ml_dtypes

    def conv_bf(a):
        return np.ascontiguousarray(np.asarray(a).astype(ml_dtypes.bfloat16))

    if use_bf16 is True:
        conv_x = conv_w2 = conv_bf
    elif use_bf16 == "hybrid":
        conv_x, conv_w2 = conv_bf, _round_f32r
    else:
        conv_x = conv_w2 = _round_f32r

    w1 = np.asarray(w1, dtype=np.float32)
    w2 = np.asarray(w2, dtype=np.float32)
    b1 = np.asarray(b1, dtype=np.float32)
    b2 = np.asarray(b2, dtype=np.float32)
    wconv = {ex: (conv_x(w1[ex]), conv_w2(w2[ex])) for ex in (0, 1)}

    in_maps = []
    core_tok = []  # per-core real token ids
    for core in range(NCORES):
        ex = core // 4
        part = core % 4
        ids = idx[ex][part * per_core : (part + 1) * per_core]
        core_tok.append(ids)
        xTc = np.zeros((D, n), dtype=np.float32)
        if len(ids):
            xTc[:, : len(ids)] = x_f[ids].T
        cl = np.zeros((1, n), dtype=np.float32)
        cl[0, : len(ids)] = c[ids, ex]
        in_maps.append(
            {
                "xT": conv_x(xTc),
                "w1": wconv[ex][0],
                "w2": wconv[ex][1],
                "b1t": np.ascontiguousarray(b1[ex].reshape(F // P, P).T.astype(np.float32)),
                "b2t": np.ascontiguousarray(b2[ex].reshape(D // P, P).T.astype(np.float32)),
                "c_row": cl,
            }
        )

    nc = _get_nc(n, use_bf16)
    kw = {}
    if _trace:
        kw = {"trace": True, "tmpdir": _tmpdir}
    res = run_bass_kernel_spmd(nc, in_maps, core_ids=list(range(NCORES)), **kw)
    kernel.last_results = res

    out = np.zeros((T, D), dtype=np.float32)
    for core in range(NCORES):
        ids = core_tok[core]
        if len(ids) == 0:
            continue
        contrib = res.results[core]["outT"][:, : len(ids)].T  # [n_real, D]
        out[ids] += contrib
    return out.reshape(B, S, D)


kernel.last_results = None



# revision 4
# speedup vs baseline: 1.0735x; 1.0735x over previous
"""MoE routing kernel for Trainium2 (8 NeuronCores, SPMD).

Math (faithful to the reference, including its quirks):
  logits = x @ gate_w + gate_b                  # [B,S,E]
  weights = softmax(logits, axis=1)             # softmax over the SEQUENCE axis
  top2 values/indices over experts; only experts 0 and 1 are ever evaluated
  (the reference loops `for ind in range(top_k)` and uses expert `ind`).
  out[t] = c0[t]*eo_0[t] + c1[t]*eo_1[t], where
  eo_e = softmax_D(gelu(x@w1[e]+b1[e]) @ w2[e] + b2[e]) and c_e[t] is the
  top-2 gate weight when expert e is in token t's top-2, else 0.

Sharding: routing + dispatch on host (0.4% of FLOPs). Only tokens whose
top-2 contains expert 0/1 are computed (~25% each). Cores 0-3 handle
expert 0's tokens, cores 4-7 expert 1's, so each core streams only one
expert's weights.

Device does the two fp16 GEMMs + gelu + exp in feature-major layout and
ships the UNNORMALIZED exp(z) back; the softmax denominator and the gate
coefficient are folded in on the host during the unshard (saves the whole
on-device normalization phase). Weights are host-packed into the exact
SBUF layout so each core needs only ~14 large contiguous DMAs.
"""

import sys

import numpy as np

sys.path.insert(0, "/opt/trn_rl_repo")

import concourse.bacc as bacc  # noqa: E402
import concourse.tile as tile  # noqa: E402
from concourse import mybir  # noqa: E402
from concourse.bass_utils import run_bass_kernel_spmd  # noqa: E402

P = 128
D = 1024
F = 4096
KD = D // P  # 8
KF = F // P  # 32
NCORES = 8
CHUNK = 512  # max matmul moving free dim
MG = 4  # F-tiles per phase-A psum group
DG = 4  # D-tiles per phase-B psum group
NWARM = 30  # HAM warm-up matmuls (128-wide)
AF = mybir.ActivationFunctionType

_CACHE = {}


def _gating_coeffs(x, gate_w, gate_b):
    """Host replica of the reference gating. Returns c[T,2] float32 where
    c[:,e] is the gate weight if expert e is in the token's top-2 else 0."""
    B, S, _ = x.shape
    x = np.asarray(x, dtype=np.float32)
    logits = x.reshape(B * S, -1) @ np.asarray(gate_w, dtype=np.float32)
    logits = logits.reshape(B, S, -1) + np.asarray(gate_b, dtype=np.float32)
    m = logits.max(axis=1, keepdims=True)
    e = np.exp(logits - m)
    w = e / e.sum(axis=1, keepdims=True)
    wf = w.reshape(B * S, -1)
    top2 = np.argsort(-wf, axis=-1, kind="stable")[:, :2]
    c = np.zeros((B * S, 2), dtype=np.float32)
    for ex in (0, 1):
        sel = (top2 == ex).any(axis=1)
        c[sel, ex] = wf[sel, ex]
    return c


def _build_nc(n):
    """Bass program for one core: n tokens, one expert, all fp16.

      h = gelu(w1.T @ x.T + b1)        # [F, n] feature-major
      p = exp(w2.T @ h + b2)           # [D, n] UNNORMALIZED; host divides

    Weight layouts (packed on host):
      w1g[p, mg*(MG*CHUNK*?)...]: per m-group of MG F-tiles, k-slabs of
        MG*128 cols:  w1g[p, mg*KD*MG*P + k*MG*P + mi*P + j]
                        = w1[k*P+p, (MG*mg+mi)*P + j]
      w2g[p, dg*KF*DG*P + kf*DG*P + di*P + j] = w2[kf*P+p, (DG*dg+di)*P+j]
      xg[p, k*n + t] = x[t, k*P + p]
    """
    dt = mybir.dt
    f16 = dt.float16
    f32 = dt.float32
    chunks = []
    off = 0
    nch = (n + CHUNK - 1) // CHUNK
    base = n // nch
    rem = n - base * nch
    for i in range(nch):
        sz = base + (1 if i < rem else 0)
        chunks.append((off, sz))
        off += sz
    NMG = KF // MG  # 8 phase-A groups
    NDG = KD // DG  # 2 phase-B groups
    GW1 = KD * MG * P  # cols per w1 group (4096)
    GW2 = KF * DG * P  # cols per w2 group (16384)

    nc = bacc.Bacc()
    xg = nc.dram_tensor("xg", [P, KD * n], f16, kind="ExternalInput")
    w1d = nc.dram_tensor("w1g", [P, NMG * GW1], f16, kind="ExternalInput")
    w2d = nc.dram_tensor("w2g", [P, NDG * GW2], f16, kind="ExternalInput")
    b1d = nc.dram_tensor("b1t", [P, KF], f32, kind="ExternalInput")
    b2d = nc.dram_tensor("b2t", [P, KD], f32, kind="ExternalInput")
    pd = nc.dram_tensor("pout", [P, KD * n], f16, kind="ExternalOutput")

    with tile.TileContext(nc) as tc:
        with (
            tc.tile_pool(name="const", bufs=1) as const,
            tc.tile_pool(name="acts", bufs=1) as acts,
            tc.tile_pool(name="w1p", bufs=NMG) as w1p,
            tc.tile_pool(name="w2p", bufs=NDG) as w2p,
            tc.tile_pool(name="ps", bufs=8, space="PSUM") as ps,
        ):
            warm_f = const.tile([P, P], f32)
            nc.vector.memset(warm_f[:], 0.0)
            warm = const.tile([P, P], f16)
            nc.vector.tensor_copy(warm[:], warm_f[:])

            # --- input DMAs: few, large, spread over sequencer queues ---
            xs = acts.tile([P, KD * n], f16)
            nc.sync.dma_start(xs[:, :n], xg[:, :n])  # k=0 first
            nc.gpsimd.dma_start(xs[:, n:], xg[:, n:])
            w1t = [w1p.tile([P, GW1], f16, tag="w1", name=f"w1_{g}") for g in range(NMG)]
            for g in range(NMG):
                eng = nc.sync if g % 2 == 0 else nc.gpsimd
                eng.dma_start(w1t[g][:], w1d[:, g * GW1 : (g + 1) * GW1])
            w2t = [w2p.tile([P, GW2], f16, tag="w2", name=f"w2_{g}") for g in range(NDG)]

            def load_w2(g):
                half = GW2 // 2
                for q in range(2):
                    nc.scalar.dma_start(
                        w2t[g][:, q * half : (q + 1) * half],
                        w2d[:, g * GW2 + q * half : g * GW2 + (q + 1) * half],
                    )

            b1t = const.tile([P, KF], f32)
            nc.scalar.dma_start(b1t[:], b1d[:])
            b2t = const.tile([P, KD], f32)
            nc.scalar.dma_start(b2t[:], b2d[:])
            load_w2(0)

            h = acts.tile([P, KF * n], f16)
            p = acts.tile([P, KD * n], f16)

            # --- HAM warm-up: prime the PE clock while DMAs land ---
            warm_ps = ps.tile([P, CHUNK], f32, tag="ps", name="warm_ps")
            for _ in range(NWARM):
                nc.tensor.matmul(warm_ps[:, :P], warm[:], warm[:], start=True, stop=True)
            warm_out = const.tile([1, 1], f32)
            nc.vector.tensor_copy(warm_out[:], warm_ps[0:1, 0:1])

            # --- Phase A: h = gelu(w1.T @ x.T + b1) ---
            for mg in range(NMG):
                pst = [
                    ps.tile([P, csz], f32, tag="ps", name=f"psa_{mg}_{mi}_{ci}")
                    for mi in range(MG)
                    for ci, (c0, csz) in enumerate(chunks)
                ]
                for k in range(KD):
                    for mi in range(MG):
                        for ci, (c0, csz) in enumerate(chunks):
                            nc.tensor.matmul(
                                pst[mi * len(chunks) + ci][:],
                                w1t[mg][:, k * MG * P + mi * P : k * MG * P + (mi + 1) * P],
                                xs[:, k * n + c0 : k * n + c0 + csz],
                                start=(k == 0),
                                stop=(k == KD - 1),
                            )
                for mi in range(MG):
                    m = MG * mg + mi
                    for ci, (c0, csz) in enumerate(chunks):
                        nc.scalar.activation(
                            h[:, m * n + c0 : m * n + c0 + csz],
                            pst[mi * len(chunks) + ci][:],
                            AF.Gelu,
                            bias=b1t[:, m : m + 1],
                        )
                if mg == 3:
                    # pace the second w2 group's 4MB load into mid-phase-A
                    load_w2(1)

            # --- Phase B: p = exp(w2.T @ h + b2), stream out per D-tile ---
            for dg in range(NDG):
                pst = [
                    ps.tile([P, csz], f32, tag="ps", name=f"psb_{dg}_{di}_{ci}")
                    for di in range(DG)
                    for ci, (c0, csz) in enumerate(chunks)
                ]
                for kf in range(KF):
                    for di in range(DG):
                        for ci, (c0, csz) in enumerate(chunks):
                            nc.tensor.matmul(
                                pst[di * len(chunks) + ci][:],
                                w2t[dg][:, kf * DG * P + di * P : kf * DG * P + (di + 1) * P],
                                h[:, kf * n + c0 : kf * n + c0 + csz],
                                start=(kf == 0),
                                stop=(kf == KF - 1),
                            )
                for di in range(DG):
                    dd = DG * dg + di
                    for ci, (c0, csz) in enumerate(chunks):
                        nc.scalar.activation(
                            p[:, dd * n + c0 : dd * n + c0 + csz],
                            pst[di * len(chunks) + ci][:],
                            AF.Exp,
                            bias=b2t[:, dd : dd + 1],
                        )
                    eng = nc.sync if di % 2 == 0 else nc.gpsimd
                    eng.dma_start(
                        pd[:, dd * n : (dd + 1) * n], p[:, dd * n : (dd + 1) * n]
                    )

    nc.finalize()
    return nc


def _get_nc(n):
    if n not in _CACHE:
        _CACHE[n] = _build_nc(n)
    return _CACHE[n]


def _pack_w1(w1e):
    # [D, F] -> [P, NMG*GW1] with w1g[p, mg*GW1 + k*MG*P + mi*P + j]
    a = w1e.reshape(KD, P, KF // MG, MG, P)  # [k, p, mg, mi, j]
    return np.ascontiguousarray(
        a.transpose(1, 2, 0, 3, 4).reshape(P, KD * KF * P).astype(np.float16)
    )


def _pack_w2(w2e):
    # [F, D] -> [P, NDG*GW2] with w2g[p, dg*GW2 + kf*DG*P + di*P + j]
    a = w2e.reshape(KF, P, KD // DG, DG, P)  # [kf, p, dg, di, j]
    return np.ascontiguousarray(
        a.transpose(1, 2, 0, 3, 4).reshape(P, KF * KD * P).astype(np.float16)
    )


def kernel(x, gate_w, gate_b, w1, b1, w2, b2, top_k, use_bf16=None,
           _trace=False, _tmpdir=None):
    x = np.asarray(x)
    B, S, _ = x.shape
    T = B * S
    assert int(top_k) == 2
    c = _gating_coeffs(x, gate_w, gate_b)

    x_f = np.ascontiguousarray(x.reshape(T, D).astype(np.float32))
    idx = [np.nonzero(c[:, ex])[0] for ex in (0, 1)]  # tokens per expert
    per_core = max((len(idx[0]) + 3) // 4, (len(idx[1]) + 3) // 4, 1)
    n = max(((per_core + 3) // 4) * 4, 64)  # small alignment only

    w1 = np.asarray(w1, dtype=np.float32)
    w2 = np.asarray(w2, dtype=np.float32)
    b1 = np.asarray(b1, dtype=np.float32)
    b2 = np.asarray(b2, dtype=np.float32)
    wconv = {ex: (_pack_w1(w1[ex]), _pack_w2(w2[ex])) for ex in (0, 1)}
    bconv = {
        ex: (
            np.ascontiguousarray(b1[ex].reshape(KF, P).T),
            np.ascontiguousarray(b2[ex].reshape(KD, P).T),
        )
        for ex in (0, 1)
    }

    in_maps = []
    core_tok = []  # per-core real token ids
    for core in range(NCORES):
        ex = core // 4
        part = core % 4
        ids = idx[ex][part * per_core : (part + 1) * per_core]
        core_tok.append(ids)
        xgc = np.zeros((D, n), dtype=np.float32)
        if len(ids):
            xgc[:, : len(ids)] = x_f[ids].T
        xgc = (
            xgc.reshape(KD, P, n).transpose(1, 0, 2).reshape(P, KD * n)
        ).astype(np.float16)
        in_maps.append(
            {
                "xg": np.ascontiguousarray(xgc),
                "w1g": wconv[ex][0],
                "w2g": wconv[ex][1],
                "b1t": bconv[ex][0],
                "b2t": bconv[ex][1],
            }
        )

    nc = _get_nc(n)
    kw = {}
    if _trace:
        kw = {"trace": True, "tmpdir": _tmpdir}
    res = run_bass_kernel_spmd(nc, in_maps, core_ids=list(range(NCORES)), **kw)
    kernel.last_results = res

    out = np.zeros((T, D), dtype=np.float32)
    for core in range(NCORES):
        ids = core_tok[core]
        if len(ids) == 0:
            continue
        ex = core // 4
        pr = res.results[core]["pout"].reshape(P, KD, n)
        p_t = (
            pr[:, :, : len(ids)].transpose(2, 1, 0).reshape(len(ids), D).astype(np.float32)
        )
        s = p_t.sum(axis=1)
        g = c[ids, ex] / s
        out[ids] += g[:, None] * p_t
    return out.reshape(B, S, D)


kernel.last_results = None


# revision 6
# speedup vs baseline: 1.2293x; 1.1452x over previous
"""MoE routing kernel for Trainium2 (8 NeuronCores, SPMD).

Math (faithful to the reference, including its quirks):
  logits = x @ gate_w + gate_b                  # [B,S,E]
  weights = softmax(logits, axis=1)             # softmax over the SEQUENCE axis
  top2 values/indices over experts; only experts 0 and 1 are ever evaluated
  (the reference loops `for ind in range(top_k)` and uses expert `ind`).
  out[t] = c0[t]*eo_0[t] + c1[t]*eo_1[t], where
  eo_e = softmax_D(gelu(x@w1[e]+b1[e]) @ w2[e] + b2[e]) and c_e[t] is the
  top-2 gate weight when expert e is in token t's top-2, else 0.

Sharding: routing + dispatch on host (0.4% of FLOPs). Only tokens whose
top-2 contains expert 0/1 are computed (~25% each). Cores 0-3 handle
expert 0's tokens, cores 4-7 expert 1's, so each core streams only one
expert's weights.

Device: phase A = fp16 GEMM + fused gelu (per-F-tile streaming, k-inner);
phase B = GEMM + fused exp, optionally fp8e4 with DoubleRow (2x PE rate,
w2 host-scaled by 512 and the 1/512 folded into the exp activation's
scale). The UNNORMALIZED exp(z) ships back; softmax denominator + gate
coefficient fold in on the host during the unshard. Weights are
host-packed into the exact SBUF layout (few large contiguous DMAs).
"""

import sys

import numpy as np

sys.path.insert(0, "/opt/trn_rl_repo")

import concourse.bacc as bacc  # noqa: E402
import concourse.tile as tile  # noqa: E402
from concourse import mybir  # noqa: E402
from concourse.bass_utils import run_bass_kernel_spmd  # noqa: E402

P = 128
D = 1024
F = 4096
KD = D // P  # 8
KF = F // P  # 32
NCORES = 8
CHUNK = 512  # max matmul moving free dim / PSUM bank width (f32)
MG = 4  # F-tiles per w1 DMA group
DG = 4  # D-tiles per w2 DMA group
NWARM = 14  # HAM warm-up matmuls (128-wide)
W2SCALE = 512.0  # fp8 phase-B weight pre-scale (undone in exp's scale)
AF = mybir.ActivationFunctionType

_CACHE = {}


def _gating_coeffs(x, gate_w, gate_b):
    """Host replica of the reference gating. Returns c[T,2] float32 where
    c[:,e] is the gate weight if expert e is in the token's top-2 else 0."""
    B, S, _ = x.shape
    x = np.asarray(x, dtype=np.float32)
    logits = x.reshape(B * S, -1) @ np.asarray(gate_w, dtype=np.float32)
    logits = logits.reshape(B, S, -1) + np.asarray(gate_b, dtype=np.float32)
    m = logits.max(axis=1, keepdims=True)
    e = np.exp(logits - m)
    w = e / e.sum(axis=1, keepdims=True)
    wf = w.reshape(B * S, -1)
    top2 = np.argsort(-wf, axis=-1, kind="stable")[:, :2]
    c = np.zeros((B * S, 2), dtype=np.float32)
    for ex in (0, 1):
        sel = (top2 == ex).any(axis=1)
        c[sel, ex] = wf[sel, ex]
    return c


def _build_nc(n, b_fp8):
    """Bass program for one core: n tokens, one expert.

      h = gelu(w1.T @ x.T + b1)        # [F, n] feature-major, fp16
      p = exp(w2.T @ h + b2)           # [D, n] UNNORMALIZED; host divides

    Weight layouts (packed on host):
      w1g[p, mg*KD*MG*P + k*MG*P + mi*P + j] = w1[k*P+p, (MG*mg+mi)*P + j]
      w2g[p, dg*KF*DG*P + kf*DG*P + di*P + j] = w2[kf*P+p, (DG*dg+di)*P+j]
      xg[p, k*n + t] = x[t, k*P + p]
    """
    dt = mybir.dt
    f16 = dt.float16
    f8 = dt.float8e4
    f32 = dt.float32
    bdt = f8 if b_fp8 else f16
    chunks = []
    off = 0
    nch = (n + CHUNK - 1) // CHUNK
    base = n // nch
    rem = n - base * nch
    for i in range(nch):
        sz = base + (1 if i < rem else 0)
        chunks.append((off, sz))
        off += sz
    NMG = KF // MG  # 8 w1 groups
    NDG = KD // DG  # 2 w2 groups
    GW1 = KD * MG * P  # cols per w1 group (4096)
    GW2 = KF * DG * P  # cols per w2 group (16384)

    nc = bacc.Bacc()
    xg = nc.dram_tensor("xg", [P, KD * n], f16, kind="ExternalInput")
    w1d = nc.dram_tensor("w1g", [P, NMG * GW1], f16, kind="ExternalInput")
    w2d = nc.dram_tensor("w2g", [P, NDG * GW2], bdt, kind="ExternalInput")
    b1d = nc.dram_tensor("b1t", [P, KF], f32, kind="ExternalInput")
    b2d = nc.dram_tensor("b2t", [P, KD], f32, kind="ExternalInput")
    pd = nc.dram_tensor("pout", [P, KD * n], f16, kind="ExternalOutput")

    with tile.TileContext(nc) as tc:
        with (
            tc.tile_pool(name="const", bufs=1) as const,
            tc.tile_pool(name="acts", bufs=1) as acts,
            tc.tile_pool(name="w1p", bufs=NMG) as w1p,
            tc.tile_pool(name="w2p", bufs=NDG) as w2p,
            tc.tile_pool(name="ps", bufs=8, space="PSUM") as ps,
        ):
            warm_f = const.tile([P, P], f32)
            nc.vector.memset(warm_f[:], 0.0)
            warm = const.tile([P, P], f16)
            nc.vector.tensor_copy(warm[:], warm_f[:])

            # --- input DMAs: few, large, spread over sequencer queues.
            # sync gets only the critical path (x k=0 + w1 group 0).
            xs = acts.tile([P, KD * n], f16)
            nc.sync.dma_start(xs[:, :n], xg[:, :n])
            w1t = [w1p.tile([P, GW1], f16, tag="w1", name=f"w1_{g}") for g in range(NMG)]
            nc.sync.dma_start(w1t[0][:], w1d[:, :GW1])
            for k in range(1, KD):
                nc.gpsimd.dma_start(xs[:, k * n : (k + 1) * n], xg[:, k * n : (k + 1) * n])
            for g in range(1, NMG):
                eng = nc.sync if g % 2 == 0 else nc.gpsimd
                eng.dma_start(w1t[g][:], w1d[:, g * GW1 : (g + 1) * GW1])
            w2t = [
                w2p.tile([P, KF, DG * P], bdt, tag="w2", name=f"w2_{g}")
                for g in range(NDG)
            ]

            def load_w2(g):
                half = KF // 2
                for q in range(2):
                    nc.scalar.dma_start(
                        w2t[g][:, q * half : (q + 1) * half, :],
                        w2d[:, g * GW2 + q * half * DG * P : g * GW2 + (q + 1) * half * DG * P],
                    )

            b1t = const.tile([P, KF], f32)
            nc.scalar.dma_start(b1t[:], b1d[:])
            b2t = const.tile([P, KD], f32)
            nc.scalar.dma_start(b2t[:], b2d[:])

            h = acts.tile([P, KF, n], f16 if not b_fp8 else f8)
            p = acts.tile([P, KD * n], f16)

            # --- HAM warm-up: prime the PE clock while the first DMAs land
            warm_ps = ps.tile([P, CHUNK], f32, tag="ps", name="warm_ps")
            for _ in range(NWARM):
                nc.tensor.matmul(warm_ps[:, :P], warm[:], warm[:], start=True, stop=True)
            warm_out = const.tile([1, 1], f32)
            nc.vector.tensor_copy(warm_out[:], warm_ps[0:1, 0:1])

            # --- Phase A: h = gelu(w1.T @ x.T + b1), fp16 ---
            # Group 0 runs k-outer so compute starts as soon as x k-slices
            # land; later groups run m-outer k-inner with per-m-tile ACT
            # streaming (psum bank frees early, ACT load spreads out).
            def act_a(m, pst):
                for ci, (c0, csz) in enumerate(chunks):
                    nc.scalar.activation(
                        h[:, m, c0 : c0 + csz],
                        pst[ci][:],
                        AF.Gelu,
                        bias=b1t[:, m : m + 1],
                    )

            g0_ps = {}
            for mi in range(MG):
                for ci, (c0, csz) in enumerate(chunks):
                    g0_ps[(mi, ci)] = ps.tile([P, csz], f32, tag="ps", name=f"psa0_{mi}_{ci}")
            for k in range(KD):
                for mi in range(MG):
                    for ci, (c0, csz) in enumerate(chunks):
                        nc.tensor.matmul(
                            g0_ps[(mi, ci)][:],
                            w1t[0][:, k * MG * P + mi * P : k * MG * P + (mi + 1) * P],
                            xs[:, k * n + c0 : k * n + c0 + csz],
                            start=(k == 0),
                            stop=(k == KD - 1),
                        )
            for mi in range(MG):
                act_a(mi, [g0_ps[(mi, ci)] for ci in range(nch)])
                if mi == 0:
                    load_w2(0)  # paced: issues after the first gelu

            for m in range(MG, KF):
                mg, mi = m // MG, m % MG
                pst = [
                    ps.tile([P, csz], f32, tag="ps", name=f"psa_{m}_{ci}")
                    for ci, (c0, csz) in enumerate(chunks)
                ]
                for k in range(KD):
                    for ci, (c0, csz) in enumerate(chunks):
                        nc.tensor.matmul(
                            pst[ci][:],
                            w1t[mg][:, k * MG * P + mi * P : k * MG * P + (mi + 1) * P],
                            xs[:, k * n + c0 : k * n + c0 + csz],
                            start=(k == 0),
                            stop=(k == KD - 1),
                        )
                act_a(m, pst)
                if m == 12:
                    load_w2(1)  # paced: second w2 group mid-phase-A

            # --- Phase B: p = exp(scale * (w2.T @ h) + b2), d-streaming ---
            kstep = 2 if b_fp8 else 1
            pmode = mybir.MatmulPerfMode.DoubleRow if b_fp8 else None
            escale = 1.0 / W2SCALE if b_fp8 else 1.0
            for d in range(KD):
                dg, di = d // DG, d % DG
                pst = [
                    ps.tile([P, csz], f32, tag="ps", name=f"psb_{d}_{ci}")
                    for ci, (c0, csz) in enumerate(chunks)
                ]
                for kf in range(0, KF, kstep):
                    for ci, (c0, csz) in enumerate(chunks):
                        if b_fp8:
                            lhsT = w2t[dg][:, kf : kf + 2, di * P : (di + 1) * P]
                            rhs = h[:, kf : kf + 2, c0 : c0 + csz]
                        else:
                            lhsT = w2t[dg][:, kf, di * P : (di + 1) * P]
                            rhs = h[:, kf, c0 : c0 + csz]
                        nc.tensor.matmul(
                            pst[ci][:],
                            lhsT,
                            rhs,
                            start=(kf == 0),
                            stop=(kf + kstep >= KF),
                            perf_mode=pmode,
                        )
                for ci, (c0, csz) in enumerate(chunks):
                    nc.scalar.activation(
                        p[:, d * n + c0 : d * n + c0 + csz],
                        pst[ci][:],
                        AF.Exp,
                        bias=b2t[:, d : d + 1],
                        scale=escale,
                    )
                eng = nc.sync if d % 2 == 0 else nc.gpsimd
                eng.dma_start(pd[:, d * n : (d + 1) * n], p[:, d * n : (d + 1) * n])

    nc.finalize()
    return nc


def _get_nc(n, b_fp8):
    key = (n, b_fp8)
    if key not in _CACHE:
        _CACHE[key] = _build_nc(n, b_fp8)
    return _CACHE[key]


def _pack_w1(w1e):
    # [D, F] -> [P, NMG*GW1] with w1g[p, mg*GW1 + k*MG*P + mi*P + j]
    a = w1e.reshape(KD, P, KF // MG, MG, P)  # [k, p, mg, mi, j]
    return np.ascontiguousarray(
        a.transpose(1, 2, 0, 3, 4).reshape(P, KD * KF * P).astype(np.float16)
    )


def _pack_w2(w2e, b_fp8):
    # [F, D] -> [P, NDG*GW2] with w2g[p, dg*GW2 + kf*DG*P + di*P + j]
    a = w2e.reshape(KF, P, KD // DG, DG, P)  # [kf, p, dg, di, j]
    a = a.transpose(1, 2, 0, 3, 4).reshape(P, KF * KD * P)
    if b_fp8:
        import ml_dtypes

        q = np.clip(a * W2SCALE, -240, 240).astype(ml_dtypes.float8_e4m3)
        return np.ascontiguousarray(q)
    return np.ascontiguousarray(a.astype(np.float16))


def kernel(x, gate_w, gate_b, w1, b1, w2, b2, top_k, use_bf16=None,
           b_fp8=True, _trace=False, _tmpdir=None):
    x = np.asarray(x)
    B, S, _ = x.shape
    T = B * S
    assert int(top_k) == 2
    c = _gating_coeffs(x, gate_w, gate_b)

    x_f = np.ascontiguousarray(x.reshape(T, D).astype(np.float32))
    idx = [np.nonzero(c[:, ex])[0] for ex in (0, 1)]  # tokens per expert
    per_core = max((len(idx[0]) + 3) // 4, (len(idx[1]) + 3) // 4, 1)
    n = max(((per_core + 15) // 16) * 16, 64)  # 16-align (fp8 DR AP stride)

    w1 = np.asarray(w1, dtype=np.float32)
    w2 = np.asarray(w2, dtype=np.float32)
    b1 = np.asarray(b1, dtype=np.float32)
    b2 = np.asarray(b2, dtype=np.float32)
    wconv = {ex: (_pack_w1(w1[ex]), _pack_w2(w2[ex], b_fp8)) for ex in (0, 1)}
    bconv = {
        ex: (
            np.ascontiguousarray(b1[ex].reshape(KF, P).T),
            np.ascontiguousarray(b2[ex].reshape(KD, P).T),
        )
        for ex in (0, 1)
    }

    in_maps = []
    core_tok = []  # per-core real token ids
    for core in range(NCORES):
        ex = core // 4
        part = core % 4
        ids = idx[ex][part * per_core : (part + 1) * per_core]
        core_tok.append(ids)
        xgc = np.zeros((D, n), dtype=np.float32)
        if len(ids):
            xgc[:, : len(ids)] = x_f[ids].T
        xgc = (
            xgc.reshape(KD, P, n).transpose(1, 0, 2).reshape(P, KD * n)
        ).astype(np.float16)
        in_maps.append(
            {
                "xg": np.ascontiguousarray(xgc),
                "w1g": wconv[ex][0],
                "w2g": wconv[ex][1],
                "b1t": bconv[ex][0],
                "b2t": bconv[ex][1],
            }
        )

    nc = _get_nc(n, b_fp8)
    kw = {}
    if _trace:
        kw = {"trace": True, "tmpdir": _tmpdir}
    res = run_bass_kernel_spmd(nc, in_maps, core_ids=list(range(NCORES)), **kw)
    kernel.last_results = res

    out = np.zeros((T, D), dtype=np.float32)
    for core in range(NCORES):
        ids = core_tok[core]
        if len(ids) == 0:
            continue
        ex = core // 4
        pr = res.results[core]["pout"].reshape(P, KD, n)
        p_t = (
            pr[:, :, : len(ids)].transpose(2, 1, 0).reshape(len(ids), D).astype(np.float32)
        )
        s = p_t.sum(axis=1)
        g = c[ids, ex] / s
        out[ids] += g[:, None] * p_t
    return out.reshape(B, S, D)


kernel.last_results = None


# revision 13
# speedup vs baseline: 1.2608x; 1.0256x over previous
"""MoE routing kernel for Trainium2 (8 NeuronCores, SPMD).

Math (faithful to the reference, including its quirks):
  logits = x @ gate_w + gate_b                  # [B,S,E]
  weights = softmax(logits, axis=1)             # softmax over the SEQUENCE axis
  top2 values/indices over experts; only experts 0 and 1 are ever evaluated
  (the reference loops `for ind in range(top_k)` and uses expert `ind`).
  out[t] = c0[t]*eo_0[t] + c1[t]*eo_1[t], where
  eo_e = softmax_D(gelu(x@w1[e]+b1[e]) @ w2[e] + b2[e]) and c_e[t] is the
  top-2 gate weight when expert e is in token t's top-2, else 0.

Sharding: routing + dispatch on host (0.4% of FLOPs). Only tokens whose
top-2 contains expert 0/1 are computed (~25% each). Cores 0-3 handle
expert 0's tokens, cores 4-7 expert 1's, so each core streams only one
expert's weights.

Device: phase A = fp16 GEMM + fused gelu (per-F-tile streaming, k-inner);
phase B = GEMM + fused exp, optionally fp8e4 with DoubleRow (2x PE rate,
w2 host-scaled by 512 and the 1/512 folded into the exp activation's
scale). The UNNORMALIZED exp(z) ships back; softmax denominator + gate
coefficient fold in on the host during the unshard. Weights are
host-packed into the exact SBUF layout (few large contiguous DMAs).
"""

import sys

import numpy as np

sys.path.insert(0, "/opt/trn_rl_repo")

import concourse.bacc as bacc  # noqa: E402
import concourse.tile as tile  # noqa: E402
from concourse import mybir  # noqa: E402
from concourse.bass_utils import run_bass_kernel_spmd  # noqa: E402

P = 128
D = 1024
F = 4096
KD = D // P  # 8
KF = F // P  # 32
NCORES = 8
CHUNK = 512  # max matmul moving free dim / PSUM bank width (f32)
MG = 4  # F-tiles per w1 DMA group
DG = 4  # D-tiles per w2 DMA group
NWARM = 10  # HAM warm-up matmuls
W2SCALE = 512.0  # fp8 phase-B weight pre-scale (undone in exp's scale)
AF = mybir.ActivationFunctionType

_CACHE = {}


def _gating_coeffs(x, gate_w, gate_b):
    """Host replica of the reference gating. Returns c[T,2] float32 where
    c[:,e] is the gate weight if expert e is in the token's top-2 else 0."""
    B, S, _ = x.shape
    x = np.asarray(x, dtype=np.float32)
    logits = x.reshape(B * S, -1) @ np.asarray(gate_w, dtype=np.float32)
    logits = logits.reshape(B, S, -1) + np.asarray(gate_b, dtype=np.float32)
    m = logits.max(axis=1, keepdims=True)
    e = np.exp(logits - m)
    w = e / e.sum(axis=1, keepdims=True)
    wf = w.reshape(B * S, -1)
    top2 = np.argsort(-wf, axis=-1, kind="stable")[:, :2]
    c = np.zeros((B * S, 2), dtype=np.float32)
    for ex in (0, 1):
        sel = (top2 == ex).any(axis=1)
        c[sel, ex] = wf[sel, ex]
    return c


def _build_nc(n, b_fp8):
    """Bass program for one core: n tokens, one expert.

      h = gelu(w1.T @ x.T + b1)        # [F, n] feature-major, fp16
      p = exp(w2.T @ h + b2)           # [D, n] UNNORMALIZED; host divides

    Weight layouts (packed on host):
      w1g[p, mg*KD*MG*P + k*MG*P + mi*P + j] = w1[k*P+p, (MG*mg+mi)*P + j]
      w2g[p, dg*KF*DG*P + kf*DG*P + di*P + j] = w2[kf*P+p, (DG*dg+di)*P+j]
      xg[p, k*n + t] = x[t, k*P + p]
    """
    dt = mybir.dt
    f16 = dt.float16
    f8 = dt.float8e4
    f32 = dt.float32
    bdt = f8 if b_fp8 else f16
    chunks = []
    off = 0
    nch = (n + CHUNK - 1) // CHUNK
    base = n // nch
    rem = n - base * nch
    for i in range(nch):
        sz = base + (1 if i < rem else 0)
        chunks.append((off, sz))
        off += sz
    NMG = KF // MG  # 8 w1 groups
    NDG = KD // DG  # 2 w2 groups
    GW1 = KD * MG * P  # cols per w1 group (4096)
    GW2 = KF * DG * P  # cols per w2 group (16384)

    nc = bacc.Bacc()
    xg = nc.dram_tensor("xg", [P, KD * n], f16, kind="ExternalInput")
    w1d = nc.dram_tensor("w1g", [P, NMG * GW1], f16, kind="ExternalInput")
    w2d = nc.dram_tensor("w2g", [P, NDG * GW2], bdt, kind="ExternalInput")
    b1d = nc.dram_tensor("b1t", [P, KF], f32, kind="ExternalInput")
    b2d = nc.dram_tensor("b2t", [P, KD], f32, kind="ExternalInput")
    pd = nc.dram_tensor("pout", [P, KD * n], f16, kind="ExternalOutput")

    with tile.TileContext(nc) as tc:
        with (
            tc.tile_pool(name="const", bufs=1) as const,
            tc.tile_pool(name="acts", bufs=1) as acts,
            tc.tile_pool(name="w1p", bufs=NMG) as w1p,
            tc.tile_pool(name="w2p", bufs=NDG) as w2p,
            tc.tile_pool(name="ps", bufs=8, space="PSUM") as ps,
        ):
            warm_f = const.tile([P, P], f32)
            nc.vector.memset(warm_f[:], 0.0)
            warm = const.tile([P, P], f16)
            nc.vector.tensor_copy(warm[:], warm_f[:])

            # --- input DMAs: few, large, spread over sequencer queues.
            # sync+gpsimd carry the critical path (x k=0 + w1, group 0
            # split across both); scalar streams the remaining x k-slices.
            xs = acts.tile([P, KD * n], f16)
            nc.sync.dma_start(xs[:, :n], xg[:, :n])
            b1t = const.tile([P, KF], f32)
            nc.sync.dma_start(b1t[:], b1d[:])
            b2t = const.tile([P, KD], f32)
            nc.sync.dma_start(b2t[:], b2d[:])
            w1t = [w1p.tile([P, GW1], f16, tag="w1", name=f"w1_{g}") for g in range(NMG)]
            nc.sync.dma_start(w1t[0][:, : GW1 // 2], w1d[:, : GW1 // 2])
            nc.gpsimd.dma_start(w1t[0][:, GW1 // 2 :], w1d[:, GW1 // 2 : GW1])
            nc.scalar.dma_start(xs[:, n : 2 * n], xg[:, n : 2 * n])
            nc.scalar.dma_start(xs[:, 2 * n : 4 * n], xg[:, 2 * n : 4 * n])
            nc.scalar.dma_start(xs[:, 4 * n :], xg[:, 4 * n :])
            for g in range(1, NMG):
                eng = nc.gpsimd if g % 2 == 1 else nc.sync
                eng.dma_start(w1t[g][:], w1d[:, g * GW1 : (g + 1) * GW1])
            w2t = [
                w2p.tile([P, KF, DG * P], bdt, tag="w2", name=f"w2_{g}")
                for g in range(NDG)
            ]

            def load_w2(g, q):
                half = KF // 2
                nc.scalar.dma_start(
                    w2t[g][:, q * half : (q + 1) * half, :],
                    w2d[:, g * GW2 + q * half * DG * P : g * GW2 + (q + 1) * half * DG * P],
                )

            h = acts.tile([P, KF, n], f16 if not b_fp8 else f8)
            p = acts.tile([P, KD * n], f16)

            # --- HAM warm-up: prime the PE clock while the first DMAs land
            warm_ps = ps.tile([P, CHUNK], f32, tag="ps", name="warm_ps")
            for _ in range(NWARM):
                nc.tensor.matmul(warm_ps[:, :P], warm[:], warm[:], start=True, stop=True)
            warm_out = const.tile([1, 1], f32)
            nc.vector.tensor_copy(warm_out[:], warm_ps[0:1, 0:1])

            # --- Phase A: h = gelu(w1.T @ x.T + b1), fp16 ---
            # Group 0 runs k-outer so compute starts as soon as x k-slices
            # land; later groups run m-outer k-inner with per-m-tile ACT
            # streaming (psum bank frees early, ACT load spreads out).
            def act_a(m, pst):
                for ci, (c0, csz) in enumerate(chunks):
                    nc.scalar.activation(
                        h[:, m, c0 : c0 + csz],
                        pst[ci][:],
                        AF.Gelu,
                        bias=b1t[:, m : m + 1],
                    )

            g0_ps = {}
            for mi in range(MG):
                for ci, (c0, csz) in enumerate(chunks):
                    g0_ps[(mi, ci)] = ps.tile([P, csz], f32, tag="ps", name=f"psa0_{mi}_{ci}")
            for k in range(KD):
                for mi in range(MG):
                    for ci, (c0, csz) in enumerate(chunks):
                        nc.tensor.matmul(
                            g0_ps[(mi, ci)][:],
                            w1t[0][:, k * MG * P + mi * P : k * MG * P + (mi + 1) * P],
                            xs[:, k * n + c0 : k * n + c0 + csz],
                            start=(k == 0),
                            stop=(k == KD - 1),
                        )
            for mi in range(MG):
                act_a(mi, [g0_ps[(mi, ci)] for ci in range(nch)])
                if mi == 0:
                    load_w2(0, 0)  # paced: issues after the first gelu

            for m in range(MG, KF):
                mg, mi = m // MG, m % MG
                pst = [
                    ps.tile([P, csz], f32, tag="ps", name=f"psa_{m}_{ci}")
                    for ci, (c0, csz) in enumerate(chunks)
                ]
                for k in range(KD):
                    for ci, (c0, csz) in enumerate(chunks):
                        nc.tensor.matmul(
                            pst[ci][:],
                            w1t[mg][:, k * MG * P + mi * P : k * MG * P + (mi + 1) * P],
                            xs[:, k * n + c0 : k * n + c0 + csz],
                            start=(k == 0),
                            stop=(k == KD - 1),
                        )
                act_a(m, pst)
                if m == 8:
                    load_w2(0, 1)
                elif m == 14:
                    load_w2(1, 0)
                elif m == 20:
                    load_w2(1, 1)

            # --- Phase B: p = exp(scale * (w2.T @ h) + b2), d-streaming ---
            kstep = 2 if b_fp8 else 1
            pmode = mybir.MatmulPerfMode.DoubleRow if b_fp8 else None
            escale = 1.0 / W2SCALE if b_fp8 else 1.0
            for d in range(KD):
                dg, di = d // DG, d % DG
                pst = [
                    ps.tile([P, csz], f32, tag="ps", name=f"psb_{d}_{ci}")
                    for ci, (c0, csz) in enumerate(chunks)
                ]
                for kf in range(0, KF, kstep):
                    for ci, (c0, csz) in enumerate(chunks):
                        if b_fp8:
                            lhsT = w2t[dg][:, kf : kf + 2, di * P : (di + 1) * P]
                            rhs = h[:, kf : kf + 2, c0 : c0 + csz]
                        else:
                            lhsT = w2t[dg][:, kf, di * P : (di + 1) * P]
                            rhs = h[:, kf, c0 : c0 + csz]
                        nc.tensor.matmul(
                            pst[ci][:],
                            lhsT,
                            rhs,
                            start=(kf == 0),
                            stop=(kf + kstep >= KF),
                            perf_mode=pmode,
                        )
                for ci, (c0, csz) in enumerate(chunks):
                    nc.scalar.activation(
                        p[:, d * n + c0 : d * n + c0 + csz],
                        pst[ci][:],
                        AF.Exp,
                        bias=b2t[:, d : d + 1],
                        scale=escale,
                    )
                eng = nc.sync if d % 2 == 0 else nc.gpsimd
                eng.dma_start(pd[:, d * n : (d + 1) * n], p[:, d * n : (d + 1) * n])

    nc.finalize()
    return nc


def _get_nc(n, b_fp8):
    key = (n, b_fp8)
    if key not in _CACHE:
        _CACHE[key] = _build_nc(n, b_fp8)
    return _CACHE[key]


def _pack_w1(w1e):
    # [D, F] -> [P, NMG*GW1] with w1g[p, mg*GW1 + k*MG*P + mi*P + j]
    a = w1e.reshape(KD, P, KF // MG, MG, P)  # [k, p, mg, mi, j]
    return np.ascontiguousarray(
        a.transpose(1, 2, 0, 3, 4).reshape(P, KD * KF * P).astype(np.float16)
    )


def _pack_w2(w2e, b_fp8):
    # [F, D] -> [P, NDG*GW2] with w2g[p, dg*GW2 + kf*DG*P + di*P + j]
    a = w2e.reshape(KF, P, KD // DG, DG, P)  # [kf, p, dg, di, j]
    a = a.transpose(1, 2, 0, 3, 4).reshape(P, KF * KD * P)
    if b_fp8:
        import ml_dtypes

        q = np.clip(a * W2SCALE, -240, 240).astype(ml_dtypes.float8_e4m3)
        return np.ascontiguousarray(q)
    return np.ascontiguousarray(a.astype(np.float16))


def kernel(x, gate_w, gate_b, w1, b1, w2, b2, top_k, use_bf16=None,
           b_fp8=True, _trace=False, _tmpdir=None):
    x = np.asarray(x)
    B, S, _ = x.shape
    T = B * S
    assert int(top_k) == 2
    c = _gating_coeffs(x, gate_w, gate_b)

    x_f = np.ascontiguousarray(x.reshape(T, D).astype(np.float32))
    idx = [np.nonzero(c[:, ex])[0] for ex in (0, 1)]  # tokens per expert
    per_core = max((len(idx[0]) + 3) // 4, (len(idx[1]) + 3) // 4, 1)
    n = max(((per_core + 15) // 16) * 16, 64)  # 16-align (fp8 DR AP stride)

    w1 = np.asarray(w1, dtype=np.float32)
    w2 = np.asarray(w2, dtype=np.float32)
    b1 = np.asarray(b1, dtype=np.float32)
    b2 = np.asarray(b2, dtype=np.float32)
    wconv = {ex: (_pack_w1(w1[ex]), _pack_w2(w2[ex], b_fp8)) for ex in (0, 1)}
    bconv = {
        ex: (
            np.ascontiguousarray(b1[ex].reshape(KF, P).T),
            np.ascontiguousarray(b2[ex].reshape(KD, P).T),
        )
        for ex in (0, 1)
    }

    in_maps = []
    core_tok = []  # per-core real token ids
    for core in range(NCORES):
        ex = core // 4
        part = core % 4
        ids = idx[ex][part * per_core : (part + 1) * per_core]
        core_tok.append(ids)
        xgc = np.zeros((D, n), dtype=np.float32)
        if len(ids):
            xgc[:, : len(ids)] = x_f[ids].T
        xgc = (
            xgc.reshape(KD, P, n).transpose(1, 0, 2).reshape(P, KD * n)
        ).astype(np.float16)
        in_maps.append(
            {
                "xg": np.ascontiguousarray(xgc),
                "w1g": wconv[ex][0],
                "w2g": wconv[ex][1],
                "b1t": bconv[ex][0],
                "b2t": bconv[ex][1],
            }
        )

    nc = _get_nc(n, b_fp8)
    kw = {}
    if _trace:
        kw = {"trace": True, "tmpdir": _tmpdir}
    res = run_bass_kernel_spmd(nc, in_maps, core_ids=list(range(NCORES)), **kw)
    kernel.last_results = res

    out = np.zeros((T, D), dtype=np.float32)
    for core in range(NCORES):
        ids = core_tok[core]
        if len(ids) == 0:
            continue
        ex = core // 4
        pr = res.results[core]["pout"].reshape(P, KD, n)
        p_t = (
            pr[:, :, : len(ids)].transpose(2, 1, 0).reshape(len(ids), D).astype(np.float32)
        )
        s = p_t.sum(axis=1)
        g = c[ids, ex] / s
        out[ids] += g[:, None] * p_t
    return out.reshape(B, S, D)


kernel.last_results = None


# revision 15
# speedup vs baseline: 1.3858x; 1.0991x over previous
"""MoE routing kernel for Trainium2 (8 NeuronCores, SPMD).

Math (faithful to the reference, including its quirks):
  logits = x @ gate_w + gate_b                  # [B,S,E]
  weights = softmax(logits, axis=1)             # softmax over the SEQUENCE axis
  top2 values/indices over experts; only experts 0 and 1 are ever evaluated
  (the reference loops `for ind in range(top_k)` and uses expert `ind`).
  out[t] = c0[t]*eo_0[t] + c1[t]*eo_1[t], where
  eo_e = softmax_D(gelu(x@w1[e]+b1[e]) @ w2[e] + b2[e]) and c_e[t] is the
  top-2 gate weight when expert e is in token t's top-2, else 0.

Sharding: routing + dispatch on host (0.4% of FLOPs). Only tokens whose
top-2 contains expert 0/1 are computed (~25% each). Cores 0-3 handle
expert 0's tokens, cores 4-7 expert 1's, so each core streams only one
expert's weights.

Device: phase A = fp16 GEMM + fused gelu (per-F-tile streaming, k-inner);
phase B = GEMM + fused exp, optionally fp8e4 with DoubleRow (2x PE rate,
w2 host-scaled by 512 and the 1/512 folded into the exp activation's
scale). The UNNORMALIZED exp(z) ships back; softmax denominator + gate
coefficient fold in on the host during the unshard. Weights are
host-packed into the exact SBUF layout (few large contiguous DMAs).
"""

import sys

import numpy as np

sys.path.insert(0, "/opt/trn_rl_repo")

import concourse.bacc as bacc  # noqa: E402
import concourse.tile as tile  # noqa: E402
from concourse import mybir  # noqa: E402
from concourse.bass_utils import run_bass_kernel_spmd  # noqa: E402

P = 128
D = 1024
F = 4096
KD = D // P  # 8
KF = F // P  # 32
NCORES = 8
CHUNK = 512  # max matmul moving free dim / PSUM bank width (f32)
MG = 4  # F-tiles per w1 DMA group
DG = 4  # D-tiles per w2 DMA group
NWARM = 10  # HAM warm-up matmuls
W2SCALE = 512.0  # fp8 phase-B weight pre-scale (undone in exp's scale)
AF = mybir.ActivationFunctionType

_CACHE = {}


def _gating_coeffs(x, gate_w, gate_b):
    """Host replica of the reference gating. Returns c[T,2] float32 where
    c[:,e] is the gate weight if expert e is in the token's top-2 else 0."""
    B, S, _ = x.shape
    x = np.asarray(x, dtype=np.float32)
    logits = x.reshape(B * S, -1) @ np.asarray(gate_w, dtype=np.float32)
    logits = logits.reshape(B, S, -1) + np.asarray(gate_b, dtype=np.float32)
    m = logits.max(axis=1, keepdims=True)
    e = np.exp(logits - m)
    w = e / e.sum(axis=1, keepdims=True)
    wf = w.reshape(B * S, -1)
    top2 = np.argsort(-wf, axis=-1, kind="stable")[:, :2]
    c = np.zeros((B * S, 2), dtype=np.float32)
    for ex in (0, 1):
        sel = (top2 == ex).any(axis=1)
        c[sel, ex] = wf[sel, ex]
    return c


def _build_nc(n, b_fp8):
    """Bass program for one core: n tokens, one expert.

      h = gelu(w1.T @ x.T + b1)        # [F, n] feature-major, fp16
      p = exp(w2.T @ h + b2)           # [D, n] UNNORMALIZED; host divides

    Weight layouts (packed on host):
      w1g[p, mg*KD*MG*P + k*MG*P + mi*P + j] = w1[k*P+p, (MG*mg+mi)*P + j]
      w2g[p, dg*KF*DG*P + kf*DG*P + di*P + j] = w2[kf*P+p, (DG*dg+di)*P+j]
      xg[p, k*n + t] = x[t, k*P + p]
    """
    dt = mybir.dt
    f16 = dt.float16
    f8 = dt.float8e4
    f32 = dt.float32
    bdt = f8 if b_fp8 else f16
    chunks = []
    off = 0
    nch = (n + CHUNK - 1) // CHUNK
    base = n // nch
    rem = n - base * nch
    for i in range(nch):
        sz = base + (1 if i < rem else 0)
        chunks.append((off, sz))
        off += sz
    NMG = KF // MG  # 8 w1 groups
    NDG = KD // DG  # 2 w2 groups
    GW1 = KD * MG * P  # cols per w1 group (4096)
    GW2 = KF * DG * P  # cols per w2 group (16384)

    nc = bacc.Bacc()
    xg = nc.dram_tensor("xg", [P, KD * n], f16, kind="ExternalInput")
    w1d = nc.dram_tensor("w1g", [P, NMG * GW1], f16, kind="ExternalInput")
    w2d = nc.dram_tensor("w2g", [P, NDG * GW2], bdt, kind="ExternalInput")
    b1d = nc.dram_tensor("b1t", [P, KF], f32, kind="ExternalInput")
    b2d = nc.dram_tensor("b2t", [P, KD], f32, kind="ExternalInput")
    pd = nc.dram_tensor("pout", [P, KD * n], f16, kind="ExternalOutput")

    assert n <= CHUNK, "pair-psum layout assumes single-chunk token axis"
    with tile.TileContext(nc) as tc:
        with (
            tc.tile_pool(name="const", bufs=1) as const,
            tc.tile_pool(name="acts", bufs=1) as acts,
            tc.tile_pool(name="wt", bufs=NMG) as wt,
            tc.tile_pool(name="ps", bufs=4, space="PSUM") as ps,
        ):
            warm = const.tile([P, P], f16)
            nc.gpsimd.memset(warm[:], 0.0)

            # --- input DMAs: few, large, spread over sequencer queues.
            # sync+gpsimd carry the critical path (x + w1, group 0 split
            # across both); scalar has the biases then the activations.
            # w2 loads are paced by buffer reuse: they land in the same
            # pool slots as w1 groups 0/1, so their DMAs wait (WAR) until
            # those groups' matmuls are done -- no head-bandwidth steal.
            xs = acts.tile([P, KD * n], f16)
            w1t = [wt.tile([P, GW1], f16, tag="w", name=f"w1_{g}") for g in range(NMG)]
            nc.sync.dma_start(xs[:, :n], xg[:, :n])
            nc.sync.dma_start(w1t[0][:, : GW1 // 2], w1d[:, : GW1 // 2])
            nc.gpsimd.dma_start(w1t[0][:, GW1 // 2 :], w1d[:, GW1 // 2 : GW1])
            nc.gpsimd.dma_start(xs[:, n : 2 * n], xg[:, n : 2 * n])
            nc.sync.dma_start(xs[:, 2 * n : 4 * n], xg[:, 2 * n : 4 * n])
            nc.gpsimd.dma_start(xs[:, 4 * n : 6 * n], xg[:, 4 * n : 6 * n])
            nc.sync.dma_start(xs[:, 6 * n :], xg[:, 6 * n :])
            for g in range(1, NMG):
                eng = nc.gpsimd if g % 2 == 1 else nc.sync
                eng.dma_start(w1t[g][:], w1d[:, g * GW1 : (g + 1) * GW1])
            b1t = const.tile([P, KF], f32)
            nc.scalar.dma_start(b1t[:], b1d[:])
            b2t = const.tile([P, KD], f32)
            nc.scalar.dma_start(b2t[:], b2d[:])
            # w2 tiles reuse w1 group 0/1 buffers (tag "w") => paced DMAs
            w2t = [
                wt.tile([P, KF, DG * P], bdt, tag="w", name=f"w2_{g}")
                for g in range(NDG)
            ]
            for g in range(NDG):
                half = KF // 2
                for q in range(2):
                    nc.scalar.dma_start(
                        w2t[g][:, q * half : (q + 1) * half, :],
                        w2d[:, g * GW2 + q * half * DG * P : g * GW2 + (q + 1) * half * DG * P],
                    )

            h = acts.tile([P, KF, n], f16 if not b_fp8 else f8)
            p = acts.tile([P, KD * n], f16)

            # --- HAM warm-up: prime the PE clock while the first DMAs land
            warm_ps = ps.tile([P, 2 * CHUNK], f32, tag="ps", name="warm_ps")
            for _ in range(NWARM):
                nc.tensor.matmul(warm_ps[:, :P], warm[:], warm[:], start=True, stop=True)
            warm_out = const.tile([1, 1], f32)
            nc.vector.tensor_copy(warm_out[:], warm_ps[0:1, 0:1])

            # --- Phase A: h = gelu(w1.T @ x.T + b1), fp16 ---
            # Pair-wide psum tiles (2 banks) halve tile/sem traffic. Group 0
            # runs k-outer so compute starts as soon as x k-slices land;
            # later pairs run m-outer k-inner with streamed per-m ACTs.
            def act_a(m, pst, half):
                nc.scalar.activation(
                    h[:, m, :],
                    pst[:, half * CHUNK : half * CHUNK + n],
                    AF.Gelu,
                    bias=b1t[:, m : m + 1],
                )

            g0_ps = [ps.tile([P, 2 * CHUNK], f32, tag="ps", name=f"psa0_{i}") for i in range(2)]
            for k in range(KD):
                for mi in range(MG):
                    nc.tensor.matmul(
                        g0_ps[mi // 2][:, (mi % 2) * CHUNK : (mi % 2) * CHUNK + n],
                        w1t[0][:, k * MG * P + mi * P : k * MG * P + (mi + 1) * P],
                        xs[:, k * n : k * n + n],
                        start=(k == 0),
                        stop=(k == KD - 1),
                    )
            for mi in range(MG):
                act_a(mi, g0_ps[mi // 2], mi % 2)

            for mp in range(2, KF // 2):
                pst = ps.tile([P, 2 * CHUNK], f32, tag="ps", name=f"psa_{mp}")
                for half in range(2):
                    m = 2 * mp + half
                    mg, mi = m // MG, m % MG
                    for k in range(KD):
                        nc.tensor.matmul(
                            pst[:, half * CHUNK : half * CHUNK + n],
                            w1t[mg][:, k * MG * P + mi * P : k * MG * P + (mi + 1) * P],
                            xs[:, k * n : k * n + n],
                            start=(k == 0),
                            stop=(k == KD - 1),
                        )
                    act_a(m, pst, half)

            # --- Phase B: p = exp(scale * (w2.T @ h) + b2), d-streaming ---
            kstep = 2 if b_fp8 else 1
            pmode = mybir.MatmulPerfMode.DoubleRow if b_fp8 else None
            escale = 1.0 / W2SCALE if b_fp8 else 1.0
            for dp in range(KD // 2):
                pst = ps.tile([P, 2 * CHUNK], f32, tag="ps", name=f"psb_{dp}")
                for half in range(2):
                    d = 2 * dp + half
                    dg, di = d // DG, d % DG
                    for kf in range(0, KF, kstep):
                        if b_fp8:
                            lhsT = w2t[dg][:, kf : kf + 2, di * P : (di + 1) * P]
                            rhs = h[:, kf : kf + 2, :]
                        else:
                            lhsT = w2t[dg][:, kf, di * P : (di + 1) * P]
                            rhs = h[:, kf, :]
                        nc.tensor.matmul(
                            pst[:, half * CHUNK : half * CHUNK + n],
                            lhsT,
                            rhs,
                            start=(kf == 0),
                            stop=(kf + kstep >= KF),
                            perf_mode=pmode,
                        )
                    nc.scalar.activation(
                        p[:, d * n : (d + 1) * n],
                        pst[:, half * CHUNK : half * CHUNK + n],
                        AF.Exp,
                        bias=b2t[:, d : d + 1],
                        scale=escale,
                    )
                eng = nc.sync if dp % 2 == 0 else nc.gpsimd
                eng.dma_start(
                    pd[:, 2 * dp * n : (2 * dp + 2) * n],
                    p[:, 2 * dp * n : (2 * dp + 2) * n],
                )

    nc.finalize()
    return nc


def _get_nc(n, b_fp8):
    key = (n, b_fp8)
    if key not in _CACHE:
        _CACHE[key] = _build_nc(n, b_fp8)
    return _CACHE[key]


def _pack_w1(w1e):
    # [D, F] -> [P, NMG*GW1] with w1g[p, mg*GW1 + k*MG*P + mi*P + j]
    a = w1e.reshape(KD, P, KF // MG, MG, P)  # [k, p, mg, mi, j]
    return np.ascontiguousarray(
        a.transpose(1, 2, 0, 3, 4).reshape(P, KD * KF * P).astype(np.float16)
    )


def _pack_w2(w2e, b_fp8):
    # [F, D] -> [P, NDG*GW2] with w2g[p, dg*GW2 + kf*DG*P + di*P + j]
    a = w2e.reshape(KF, P, KD // DG, DG, P)  # [kf, p, dg, di, j]
    a = a.transpose(1, 2, 0, 3, 4).reshape(P, KF * KD * P)
    if b_fp8:
        import ml_dtypes

        q = np.clip(a * W2SCALE, -240, 240).astype(ml_dtypes.float8_e4m3)
        return np.ascontiguousarray(q)
    return np.ascontiguousarray(a.astype(np.float16))


def kernel(x, gate_w, gate_b, w1, b1, w2, b2, top_k, use_bf16=None,
           b_fp8=True, _trace=False, _tmpdir=None):
    x = np.asarray(x)
    B, S, _ = x.shape
    T = B * S
    assert int(top_k) == 2
    c = _gating_coeffs(x, gate_w, gate_b)

    x_f = np.ascontiguousarray(x.reshape(T, D).astype(np.float32))
    idx = [np.nonzero(c[:, ex])[0] for ex in (0, 1)]  # tokens per expert
    per_core = max((len(idx[0]) + 3) // 4, (len(idx[1]) + 3) // 4, 1)
    n = max(((per_core + 15) // 16) * 16, 64)  # 16-align (fp8 DR AP stride)

    w1 = np.asarray(w1, dtype=np.float32)
    w2 = np.asarray(w2, dtype=np.float32)
    b1 = np.asarray(b1, dtype=np.float32)
    b2 = np.asarray(b2, dtype=np.float32)
    wconv = {ex: (_pack_w1(w1[ex]), _pack_w2(w2[ex], b_fp8)) for ex in (0, 1)}
    bconv = {
        ex: (
            np.ascontiguousarray(b1[ex].reshape(KF, P).T),
            np.ascontiguousarray(b2[ex].reshape(KD, P).T),
        )
        for ex in (0, 1)
    }

    in_maps = []
    core_tok = []  # per-core real token ids
    for core in range(NCORES):
        ex = core // 4
        part = core % 4
        ids = idx[ex][part * per_core : (part + 1) * per_core]
        core_tok.append(ids)
        xgc = np.zeros((D, n), dtype=np.float32)
        if len(ids):
            xgc[:, : len(ids)] = x_f[ids].T
        xgc = (
            xgc.reshape(KD, P, n).transpose(1, 0, 2).reshape(P, KD * n)
        ).astype(np.float16)
        in_maps.append(
            {
                "xg": np.ascontiguousarray(xgc),
                "w1g": wconv[ex][0],
                "w2g": wconv[ex][1],
                "b1t": bconv[ex][0],
                "b2t": bconv[ex][1],
            }
        )

    nc = _get_nc(n, b_fp8)
    kw = {}
    if _trace:
        kw = {"trace": True, "tmpdir": _tmpdir}
    res = run_bass_kernel_spmd(nc, in_maps, core_ids=list(range(NCORES)), **kw)
    kernel.last_results = res

    out = np.zeros((T, D), dtype=np.float32)
    for core in range(NCORES):
        ids = core_tok[core]
        if len(ids) == 0:
            continue
        ex = core // 4
        pr = res.results[core]["pout"].reshape(P, KD, n)
        p_t = (
            pr[:, :, : len(ids)].transpose(2, 1, 0).reshape(len(ids), D).astype(np.float32)
        )
        s = p_t.sum(axis=1)
        g = c[ids, ex] / s
        out[ids] += g[:, None] * p_t
    return out.reshape(B, S, D)


kernel.last_results = None
